# revision 16
# baseline (speedup 1.0000x reference)
"""BlockAttentionResidual Trainium2 kernel.

Math (per token t, feature dim D=1024, over N+1=9 blocks):
    ssq[n,t]  = sum_d v[n,t,d]^2
    rq[n,t]   = (ssq/D + eps)^(-1/2)        (computed as exp(-0.5*ln(ssq/D+eps)))
    logit     = (sum_d w2[d]*v[n,t,d]) * rq      where w2 = proj_w*norm_w
    w[n,t]    = softmax over n of logit
    h[t,d]    = sum_n w[n,t] * v[n,t,d]

Sharding: B*T = 8192 tokens split evenly across 8 cores (1024 tokens/core).

Host-side prep: per core the 9 blocks are pre-interleaved into
vstack[quad, p, (g,d)] where partition p = 14*n + t' stacks the 9 blocks of
14 tokens (126 rows) and the free dim holds 4 such token-groups (one PSUM
page worth = 56 tokens per "quad"). This makes each quad's input a single
contiguous [126, 4096] DMA with 16KB-per-partition descriptors.

Per-quad on-chip pipeline:
  - ssq:  ScalarE activation(Square) with accum_out       (1 pass)
  - dot:  VectorE scalar_tensor_tensor mult/mult accum    (1 pass)
  - softmax over n: TensorE matmuls against a 0/1 mask M[p,t'] = (p%14==t')
      Z = M^T @ exp(logits), and M @ (1/Z) broadcasts 1/Z back to rows.
  - h:    TensorE matmul  h[t',d] = sum_p lhsT[p,t'] * v[p,d]
      with lhsT = M * w_col, 4 groups packed into one [128,1024] PSUM page
      at partition offsets 0/32/64/96 (PE column-group tiling).
  - PSUM -> SBUF copy split between VectorE and ScalarE, then DMA out.
"""

import os
import sys
import numpy as np

for _p in ("/opt/trn_rl_repo", "/root/.axon_site/_ro/trn_rl_repo"):
    if os.path.isdir(_p) and _p not in sys.path:
        sys.path.append(_p)

N_CORES = 8
N, B, T, D = 8, 4, 2048, 1024
EPS = 1e-6
TOK = (B * T) // N_CORES          # 1024 tokens per core
NB = N + 1                        # 9 stacked blocks
GROUP = 14                        # tokens per group (14*9 = 126 <= 128)
ROWS = GROUP * NB                 # 126 used partitions
QG = 8                            # groups per oct (two PSUM pages)
PAGES = QG // 4                   # PSUM pages per oct
QTOK = GROUP * QG                 # 112 tokens per oct
NQUAD = (TOK + QTOK - 1) // QTOK  # 10 (last oct ragged: 16 real tokens)

DVE_COPY_COLS = int(os.environ.get("BLOCKATTN_DVE_COPY", "560"))
ACT_SET = "natural_log_exp_and_others"

_CACHE = {}


def _groups(q):
    """[(g, t0, tg)] active groups of quad q (t0 = core-local token base)."""
    out = []
    for g in range(QG):
        t0 = q * QTOK + g * GROUP
        tg = min(GROUP, TOK - t0)
        if tg > 0:
            out.append((g, t0, tg))
    return out


def _patch_act_tables():
    """Make every activation func this kernel uses resolve to one table set
    (ACT_SET), so bacc emits a single ACT_TABLE_LOAD instead of thrashing
    between sets on every Ln/Exp/Square transition."""
    import concourse.bacc as bacc_mod
    import concourse.hw_specs as hw_specs
    from concourse import mybir

    if getattr(bacc_mod, "_blockattn_act_patch", False):
        return
    AF = mybir.ActivationFunctionType
    mine = {AF.Square, AF.Exp, AF.Ln, AF.Copy, AF.Identity}
    orig = hw_specs.get_activation_tables

    def patched(arch):
        t = dict(orig(arch))
        assert ACT_SET in t and mine <= t[ACT_SET], (ACT_SET, t.get(ACT_SET))
        return {
            name: (funcs if name == ACT_SET else funcs - mine)
            for name, funcs in t.items()
        }

    bacc_mod.get_activation_tables = patched
    bacc_mod._blockattn_act_patch = True


def build_nc():
    import concourse.bacc as bacc
    import concourse.tile as tile
    from concourse import mybir

    _patch_act_tables()

    f32 = mybir.dt.float32
    AF = mybir.ActivationFunctionType
    OP = mybir.AluOpType

    nc = bacc.Bacc("TRN2", target_bir_lowering=False, debug=False)

    vst_d = nc.dram_tensor("vstack", [NQUAD, ROWS, QG * D], f32,
                           kind="ExternalInput")
    w2b_d = nc.dram_tensor("w2b", [ROWS, D], f32, kind="ExternalInput")
    oh_d = nc.dram_tensor("onehot", [ROWS, GROUP], f32, kind="ExternalInput")
    ohT_d = nc.dram_tensor("onehotT", [GROUP, ROWS], f32, kind="ExternalInput")
    h_d = nc.dram_tensor("h", [TOK, D], f32, kind="ExternalOutput")

    vst = vst_d.ap()
    hout = h_d.ap()

    with tile.TileContext(nc) as tc:
        import contextlib
        ctx = contextlib.ExitStack()
        with ctx:
            consts = ctx.enter_context(tc.tile_pool(name="consts", bufs=1))
            vq_pool = ctx.enter_context(tc.tile_pool(name="vq", bufs=4))
            scr_pool = ctx.enter_context(tc.tile_pool(name="scr", bufs=2))
            stats_pool = ctx.enter_context(tc.tile_pool(name="stats", bufs=4))
            small_pool = ctx.enter_context(tc.tile_pool(name="small", bufs=3))
            hsb_pool = ctx.enter_context(tc.tile_pool(name="hsb", bufs=3))
            hpage_pool = ctx.enter_context(
                tc.tile_pool(name="hpage", bufs=3, space="PSUM"))
            zp_pool = ctx.enter_context(
                tc.tile_pool(name="zp", bufs=1, space="PSUM"))
            rzb_pool = ctx.enter_context(
                tc.tile_pool(name="rzb", bufs=1, space="PSUM"))

            w2b = consts.tile([ROWS, D], f32)
            nc.sync.dma_start(w2b[:], w2b_d.ap()[:])
            oh = consts.tile([ROWS, GROUP], f32)
            nc.sync.dma_start(oh[:], oh_d.ap()[:])
            ohT = consts.tile([GROUP, ROWS], f32)
            nc.sync.dma_start(ohT[:], ohT_d.ap()[:])
            zero_col = consts.tile([ROWS, 1], f32)
            nc.vector.memset(zero_col[:], 0.0)
            eps_col = consts.tile([ROWS, 1], f32)
            nc.vector.memset(eps_col[:], EPS)

            for q in range(NQUAD):
                groups = _groups(q)

                vq = vq_pool.tile([ROWS, QG * D], f32)
                stats = stats_pool.tile([ROWS, 2 * QG], f32)

                # ---- input DMA: one contiguous slab per oct ----
                nc.sync.dma_start(vq[:, :], vst[q])

                # ---- per-group stats (one full pass each on ACT and DVE) ----
                for g, t0, tg in groups:
                    gc = g * D
                    sq_scr = scr_pool.tile([ROWS, D], f32, tag="sq_scr")
                    nc.scalar.activation(
                        sq_scr[0:ROWS, :], vq[0:ROWS, gc:gc + D], AF.Square,
                        bias=zero_col[:], accum_out=stats[:, g:g + 1])
                    u_scr = scr_pool.tile([ROWS, D], f32, tag="u_scr")
                    nc.vector.scalar_tensor_tensor(
                        out=u_scr[0:ROWS, :], in0=vq[0:ROWS, gc:gc + D],
                        scalar=1.0, in1=w2b[0:ROWS, :],
                        op0=OP.mult, op1=OP.mult,
                        accum_out=stats[:, QG + g:QG + g + 1])

                # ---- softmax small ops on [126, 4] stats ----
                lnq = small_pool.tile([ROWS, QG], f32, tag="lnq")
                nc.scalar.activation(lnq[:], stats[:, 0:QG], AF.Ln,
                                     bias=eps_col[:], scale=1.0 / D)
                rq = small_pool.tile([ROWS, QG], f32, tag="rq")
                nc.scalar.activation(rq[:], lnq[:], AF.Exp,
                                     bias=zero_col[:], scale=-0.5)
                lg = small_pool.tile([ROWS, QG], f32, tag="lg")
                nc.vector.tensor_mul(lg[:], stats[:, QG:2 * QG], rq[:])
                e_sb = small_pool.tile([ROWS, QG], f32, tag="e_sb")
                nc.scalar.activation(e_sb[:], lg[:], AF.Exp, bias=zero_col[:])

                zp = zp_pool.tile([GROUP, QG], f32)
                nc.tensor.matmul(zp[:], lhsT=oh[:], rhs=e_sb[:],
                                 start=True, stop=True)
                rz = small_pool.tile([GROUP, QG], f32, tag="rz")
                nc.vector.reciprocal(rz[:], zp[:])
                rzb = rzb_pool.tile([ROWS, QG], f32)
                nc.tensor.matmul(rzb[:], lhsT=ohT[:], rhs=rz[:],
                                 start=True, stop=True)
                wcol = small_pool.tile([ROWS, QG], f32, tag="wcol")
                nc.vector.tensor_mul(wcol[:], e_sb[:], rzb[:])

                # ---- weighted sum via PE, 4 groups per PSUM page ----
                lhsTs = small_pool.tile([ROWS, QG * GROUP], f32, tag="lhsTs")
                active_pages = sorted({g // 4 for g, _, _ in groups})
                hpages = {pg: hpage_pool.tile([128, D], f32, tag="hpage",
                                              name="hpage")
                          for pg in active_pages}
                for g, t0, tg in groups:
                    gc = g * D
                    lw = lhsTs[:, g * GROUP:(g + 1) * GROUP]
                    nc.vector.tensor_scalar(
                        out=lw, in0=oh[:], scalar1=wcol[:, g:g + 1],
                        scalar2=None, op0=OP.mult)
                    pg = g // 4
                    col = 32 * (g % 4)
                    for hh in range(2):
                        nc.tensor.matmul(
                            hpages[pg][col:col + GROUP,
                                       512 * hh:512 * hh + 512],
                            lhsT=lw,
                            rhs=vq[0:ROWS, gc + 512 * hh:gc + 512 * hh + 512],
                            start=True, stop=True,
                            tile_position=(0, col))

                # ---- PSUM -> SBUF (split across DVE and ACT) -> HBM ----
                for pg in active_pages:
                    h_sb = hsb_pool.tile([128, D], f32, tag="h_sb")
                    nc.vector.tensor_copy(h_sb[:, 0:DVE_COPY_COLS],
                                          hpages[pg][:, 0:DVE_COPY_COLS])
                    nc.scalar.copy(h_sb[:, DVE_COPY_COLS:D],
                                   hpages[pg][:, DVE_COPY_COLS:D])
                    for g, t0, tg in groups:
                        if g // 4 != pg:
                            continue
                        nc.gpsimd.dma_start(hout[t0:t0 + tg, :],
                                            h_sb[32 * (g % 4):32 * (g % 4) + tg, :])

    nc.compile()
    return nc


def _host_inputs(blocks, partial_block, proj_w, norm_w):
    """Slice + interleave per-core inputs (host-side, numpy only)."""
    blocks = np.ascontiguousarray(blocks, dtype=np.float32).reshape(N, B * T, D)
    partial = np.ascontiguousarray(partial_block, dtype=np.float32).reshape(B * T, D)
    w2 = (np.asarray(proj_w, np.float32) * np.asarray(norm_w, np.float32))
    w2b = np.ascontiguousarray(np.broadcast_to(w2, (ROWS, D)), np.float32)
    oh = np.zeros((ROWS, GROUP), np.float32)
    for p in range(ROWS):
        oh[p, p % GROUP] = 1.0
    ohT = np.ascontiguousarray(oh.T)

    pad_tok = NQUAD * QTOK  # 1064
    in_maps = []
    for c in range(N_CORES):
        s = slice(c * TOK, (c + 1) * TOK)
        av = np.zeros((NB, pad_tok, D), np.float32)
        av[:N, :TOK] = blocks[:, s, :]
        av[N, :TOK] = partial[s, :]
        # vstack[q, 14n+t', g*D+d] = av[n, q*56 + g*14 + t', d]
        vst = av.reshape(NB, NQUAD, QG, GROUP, D)
        vst = np.ascontiguousarray(vst.transpose(1, 0, 3, 2, 4))
        vst = vst.reshape(NQUAD, ROWS, QG * D)
        in_maps.append({
            "vstack": vst,
            "w2b": w2b,
            "onehot": oh,
            "onehotT": ohT,
        })
    return in_maps


def kernel(blocks, partial_block, proj_w, norm_w):
    from concourse.bass_utils import run_bass_kernel_spmd

    if "nc" not in _CACHE:
        _CACHE["nc"] = build_nc()
    nc = _CACHE["nc"]
    in_maps = _host_inputs(blocks, partial_block, proj_w, norm_w)
    res = run_bass_kernel_spmd(nc, in_maps, core_ids=list(range(N_CORES)))
    h = np.concatenate([res.results[c]["h"] for c in range(N_CORES)], axis=0)
    return h.reshape(B, T, D)


# revision 19
# speedup vs baseline: 1.0184x; 1.0184x over previous
"""BlockAttentionResidual Trainium2 kernel.

Math (per token t, feature dim D=1024, over N+1=9 blocks):
    ssq[n,t]  = sum_d v[n,t,d]^2
    rq[n,t]   = (ssq/D + eps)^(-1/2)        (computed as exp(-0.5*ln(ssq/D+eps)))
    logit     = (sum_d w2[d]*v[n,t,d]) * rq      where w2 = proj_w*norm_w
    w[n,t]    = softmax over n of logit
    h[t,d]    = sum_n w[n,t] * v[n,t,d]

Sharding: B*T = 8192 tokens split evenly across 8 cores (1024 tokens/core).

Host-side prep: per core the 9 blocks are pre-interleaved into
vstack[quad, p, (g,d)] where partition p = 14*n + t' stacks the 9 blocks of
14 tokens (126 rows) and the free dim holds 4 such token-groups (one PSUM
page worth = 56 tokens per "quad"). This makes each quad's input a single
contiguous [126, 4096] DMA with 16KB-per-partition descriptors.

Per-quad on-chip pipeline:
  - ssq:  ScalarE activation(Square) with accum_out       (1 pass)
  - dot:  VectorE scalar_tensor_tensor mult/mult accum    (1 pass)
  - softmax over n: TensorE matmuls against a 0/1 mask M[p,t'] = (p%14==t')
      Z = M^T @ exp(logits), and M @ (1/Z) broadcasts 1/Z back to rows.
  - h:    TensorE matmul  h[t',d] = sum_p lhsT[p,t'] * v[p,d]
      with lhsT = M * w_col, 4 groups packed into one [128,1024] PSUM page
      at partition offsets 0/32/64/96 (PE column-group tiling).
  - PSUM -> SBUF copy split between VectorE and ScalarE, then DMA out.
"""

import os
import sys
import numpy as np

for _p in ("/opt/trn_rl_repo", "/root/.axon_site/_ro/trn_rl_repo"):
    if os.path.isdir(_p) and _p not in sys.path:
        sys.path.append(_p)

N_CORES = 8
N, B, T, D = 8, 4, 2048, 1024
EPS = 1e-6
TOK = (B * T) // N_CORES          # 1024 tokens per core
NB = N + 1                        # 9 stacked blocks
GROUP = 14                        # tokens per group (14*9 = 126 <= 128)
ROWS = GROUP * NB                 # 126 used partitions
QG = 16                           # groups per super-tile (four PSUM pages)
PAGES = QG // 4                   # PSUM pages per super-tile
QTOK = GROUP * QG                 # 224 tokens per super-tile
NQUAD = (TOK + QTOK - 1) // QTOK  # 5 (last one ragged: 128 real tokens)

DVE_COPY_COLS = int(os.environ.get("BLOCKATTN_DVE_COPY", "560"))
ACT_SET = "natural_log_exp_and_others"

_CACHE = {}


def _groups(q):
    """[(g, t0, tg)] active groups of quad q (t0 = core-local token base)."""
    out = []
    for g in range(QG):
        t0 = q * QTOK + g * GROUP
        tg = min(GROUP, TOK - t0)
        if tg > 0:
            out.append((g, t0, tg))
    return out


def _patch_act_tables():
    """Make every activation func this kernel uses resolve to one table set
    (ACT_SET), so bacc emits a single ACT_TABLE_LOAD instead of thrashing
    between sets on every Ln/Exp/Square transition."""
    import concourse.bacc as bacc_mod
    import concourse.hw_specs as hw_specs
    from concourse import mybir

    if getattr(bacc_mod, "_blockattn_act_patch", False):
        return
    AF = mybir.ActivationFunctionType
    mine = {AF.Square, AF.Exp, AF.Ln, AF.Copy, AF.Identity}
    orig = hw_specs.get_activation_tables

    def patched(arch):
        t = dict(orig(arch))
        assert ACT_SET in t and mine <= t[ACT_SET], (ACT_SET, t.get(ACT_SET))
        return {
            name: (funcs if name == ACT_SET else funcs - mine)
            for name, funcs in t.items()
        }

    bacc_mod.get_activation_tables = patched
    bacc_mod._blockattn_act_patch = True


def build_nc():
    import concourse.bacc as bacc
    import concourse.tile as tile
    from concourse import mybir

    _patch_act_tables()

    f32 = mybir.dt.float32
    AF = mybir.ActivationFunctionType
    OP = mybir.AluOpType

    nc = bacc.Bacc("TRN2", target_bir_lowering=False, debug=False)

    vst_d = nc.dram_tensor("vstack", [NQUAD, ROWS, QG * D], f32,
                           kind="ExternalInput")
    w2b_d = nc.dram_tensor("w2b", [ROWS, D], f32, kind="ExternalInput")
    oh_d = nc.dram_tensor("onehot", [ROWS, GROUP], f32, kind="ExternalInput")
    ohT_d = nc.dram_tensor("onehotT", [GROUP, ROWS], f32, kind="ExternalInput")
    h_d = nc.dram_tensor("h", [TOK, D], f32, kind="ExternalOutput")

    vst = vst_d.ap()
    hout = h_d.ap()

    with tile.TileContext(nc) as tc:
        import contextlib
        ctx = contextlib.ExitStack()
        with ctx:
            consts = ctx.enter_context(tc.tile_pool(name="consts", bufs=1))
            vq_pool = ctx.enter_context(tc.tile_pool(name="vq", bufs=2))
            scr_pool = ctx.enter_context(tc.tile_pool(name="scr", bufs=2))
            stats_pool = ctx.enter_context(tc.tile_pool(name="stats", bufs=4))
            small_pool = ctx.enter_context(tc.tile_pool(name="small", bufs=3))
            hsb_pool = ctx.enter_context(tc.tile_pool(name="hsb", bufs=3))
            hpage_pool = ctx.enter_context(
                tc.tile_pool(name="hpage", bufs=3, space="PSUM"))
            zp_pool = ctx.enter_context(
                tc.tile_pool(name="zp", bufs=1, space="PSUM"))
            rzb_pool = ctx.enter_context(
                tc.tile_pool(name="rzb", bufs=1, space="PSUM"))

            w2b = consts.tile([ROWS, D], f32)
            nc.sync.dma_start(w2b[:], w2b_d.ap()[:])
            oh = consts.tile([ROWS, GROUP], f32)
            nc.sync.dma_start(oh[:], oh_d.ap()[:])
            ohT = consts.tile([GROUP, ROWS], f32)
            nc.sync.dma_start(ohT[:], ohT_d.ap()[:])
            zero_col = consts.tile([ROWS, 1], f32)
            nc.vector.memset(zero_col[:], 0.0)
            eps_col = consts.tile([ROWS, 1], f32)
            nc.vector.memset(eps_col[:], EPS)

            for q in range(NQUAD):
                groups = _groups(q)

                vq = vq_pool.tile([ROWS, QG * D], f32)
                stats = stats_pool.tile([ROWS, 2 * QG], f32)

                # ---- input DMA: one contiguous slab per oct ----
                # (tail oct: only transfer the columns of active groups)
                used = len(groups) * D
                nc.sync.dma_start(vq[:, 0:used], vst[q][:, 0:used])

                # ---- per-group stats (one full pass each on ACT and DVE) ----
                for g, t0, tg in groups:
                    gc = g * D
                    sq_scr = scr_pool.tile([ROWS, D], f32, tag="sq_scr")
                    nc.scalar.activation(
                        sq_scr[0:ROWS, :], vq[0:ROWS, gc:gc + D], AF.Square,
                        bias=zero_col[:], accum_out=stats[:, g:g + 1])
                    u_scr = scr_pool.tile([ROWS, D], f32, tag="u_scr")
                    nc.vector.scalar_tensor_tensor(
                        out=u_scr[0:ROWS, :], in0=vq[0:ROWS, gc:gc + D],
                        scalar=1.0, in1=w2b[0:ROWS, :],
                        op0=OP.mult, op1=OP.mult,
                        accum_out=stats[:, QG + g:QG + g + 1])

                # ---- softmax small ops on [126, 4] stats ----
                lnq = small_pool.tile([ROWS, QG], f32, tag="lnq")
                nc.scalar.activation(lnq[:], stats[:, 0:QG], AF.Ln,
                                     bias=eps_col[:], scale=1.0 / D)
                rq = small_pool.tile([ROWS, QG], f32, tag="rq")
                nc.scalar.activation(rq[:], lnq[:], AF.Exp,
                                     bias=zero_col[:], scale=-0.5)
                lg = small_pool.tile([ROWS, QG], f32, tag="lg")
                nc.vector.tensor_mul(lg[:], stats[:, QG:2 * QG], rq[:])
                e_sb = small_pool.tile([ROWS, QG], f32, tag="e_sb")
                nc.scalar.activation(e_sb[:], lg[:], AF.Exp, bias=zero_col[:])

                zp = zp_pool.tile([GROUP, QG], f32)
                nc.tensor.matmul(zp[:], lhsT=oh[:], rhs=e_sb[:],
                                 start=True, stop=True)
                rz = small_pool.tile([GROUP, QG], f32, tag="rz")
                nc.vector.reciprocal(rz[:], zp[:])
                rzb = rzb_pool.tile([ROWS, QG], f32)
                nc.tensor.matmul(rzb[:], lhsT=ohT[:], rhs=rz[:],
                                 start=True, stop=True)
                wcol = small_pool.tile([ROWS, QG], f32, tag="wcol")
                nc.vector.tensor_mul(wcol[:], e_sb[:], rzb[:])

                # ---- weighted sum via PE, 4 groups per PSUM page ----
                lhsTs = small_pool.tile([ROWS, QG * GROUP], f32, tag="lhsTs")
                active_pages = sorted({g // 4 for g, _, _ in groups})
                hpages = {pg: hpage_pool.tile([128, D], f32, tag="hpage",
                                              name="hpage")
                          for pg in active_pages}
                for g, t0, tg in groups:
                    gc = g * D
                    lw = lhsTs[:, g * GROUP:(g + 1) * GROUP]
                    nc.vector.tensor_scalar(
                        out=lw, in0=oh[:], scalar1=wcol[:, g:g + 1],
                        scalar2=None, op0=OP.mult)
                    pg = g // 4
                    col = 32 * (g % 4)
                    for hh in range(2):
                        nc.tensor.matmul(
                            hpages[pg][col:col + GROUP,
                                       512 * hh:512 * hh + 512],
                            lhsT=lw,
                            rhs=vq[0:ROWS, gc + 512 * hh:gc + 512 * hh + 512],
                            start=True, stop=True,
                            tile_position=(0, col))

                # ---- PSUM -> SBUF (split across DVE and ACT) -> HBM ----
                for pg in active_pages:
                    h_sb = hsb_pool.tile([128, D], f32, tag="h_sb")
                    nc.vector.tensor_copy(h_sb[:, 0:DVE_COPY_COLS],
                                          hpages[pg][:, 0:DVE_COPY_COLS])
                    nc.scalar.copy(h_sb[:, DVE_COPY_COLS:D],
                                   hpages[pg][:, DVE_COPY_COLS:D])
                    for g, t0, tg in groups:
                        if g // 4 != pg:
                            continue
                        nc.gpsimd.dma_start(hout[t0:t0 + tg, :],
                                            h_sb[32 * (g % 4):32 * (g % 4) + tg, :])

    nc.compile()
    return nc


def _host_inputs(blocks, partial_block, proj_w, norm_w):
    """Slice + interleave per-core inputs (host-side, numpy only)."""
    blocks = np.ascontiguousarray(blocks, dtype=np.float32).reshape(N, B * T, D)
    partial = np.ascontiguousarray(partial_block, dtype=np.float32).reshape(B * T, D)
    w2 = (np.asarray(proj_w, np.float32) * np.asarray(norm_w, np.float32))
    w2b = np.ascontiguousarray(np.broadcast_to(w2, (ROWS, D)), np.float32)
    oh = np.zeros((ROWS, GROUP), np.float32)
    for p in range(ROWS):
        oh[p, p % GROUP] = 1.0
    ohT = np.ascontiguousarray(oh.T)

    pad_tok = NQUAD * QTOK  # 1064
    in_maps = []
    for c in range(N_CORES):
        s = slice(c * TOK, (c + 1) * TOK)
        av = np.zeros((NB, pad_tok, D), np.float32)
        av[:N, :TOK] = blocks[:, s, :]
        av[N, :TOK] = partial[s, :]
        # vstack[q, 14n+t', g*D+d] = av[n, q*56 + g*14 + t', d]
        vst = av.reshape(NB, NQUAD, QG, GROUP, D)
        vst = np.ascontiguousarray(vst.transpose(1, 0, 3, 2, 4))
        vst = vst.reshape(NQUAD, ROWS, QG * D)
        in_maps.append({
            "vstack": vst,
            "w2b": w2b,
            "onehot": oh,
            "onehotT": ohT,
        })
    return in_maps


def kernel(blocks, partial_block, proj_w, norm_w):
    from concourse.bass_utils import run_bass_kernel_spmd

    if "nc" not in _CACHE:
        _CACHE["nc"] = build_nc()
    nc = _CACHE["nc"]
    in_maps = _host_inputs(blocks, partial_block, proj_w, norm_w)
    res = run_bass_kernel_spmd(nc, in_maps, core_ids=list(range(N_CORES)))
    h = np.concatenate([res.results[c]["h"] for c in range(N_CORES)], axis=0)
    return h.reshape(B, T, D)


# revision 23
# speedup vs baseline: 1.1468x; 1.1261x over previous
"""BlockAttentionResidual Trainium2 kernel.

Math (per token t, feature dim D=1024, over N+1=9 blocks):
    ssq[n,t]  = sum_d v[n,t,d]^2
    rq[n,t]   = (ssq/D + eps)^(-1/2)        (computed as exp(-0.5*ln(ssq/D+eps)))
    logit     = (sum_d w2[d]*v[n,t,d]) * rq      where w2 = proj_w*norm_w
    w[n,t]    = softmax over n of logit
    h[t,d]    = sum_n w[n,t] * v[n,t,d]

Sharding: B*T = 8192 tokens split evenly across 8 cores (1024 tokens/core).

Host-side prep: per core the 9 blocks are pre-interleaved into
vstack[quad, p, (g,d)] where partition p = 14*n + t' stacks the 9 blocks of
14 tokens (126 rows) and the free dim holds 4 such token-groups (one PSUM
page worth = 56 tokens per "quad"). This makes each quad's input a single
contiguous [126, 4096] DMA with 16KB-per-partition descriptors.

Per-quad on-chip pipeline:
  - ssq:  ScalarE activation(Square) with accum_out       (1 pass)
  - dot:  VectorE scalar_tensor_tensor mult/mult accum    (1 pass)
  - softmax over n: TensorE matmuls against a 0/1 mask M[p,t'] = (p%14==t')
      Z = M^T @ exp(logits), and M @ (1/Z) broadcasts 1/Z back to rows.
  - h:    TensorE matmul  h[t',d] = sum_p lhsT[p,t'] * v[p,d]
      with lhsT = M * w_col, 4 groups packed into one [128,1024] PSUM page
      at partition offsets 0/32/64/96 (PE column-group tiling).
  - PSUM -> SBUF copy split between VectorE and ScalarE, then DMA out.
"""

import os
import sys
import numpy as np

for _p in ("/opt/trn_rl_repo", "/root/.axon_site/_ro/trn_rl_repo"):
    if os.path.isdir(_p) and _p not in sys.path:
        sys.path.append(_p)

N_CORES = 8
N, B, T, D = 8, 4, 2048, 1024
EPS = 1e-6
TOK = (B * T) // N_CORES          # 1024 tokens per core
NB = N + 1                        # 9 stacked blocks
GROUP = 14                        # tokens per group (14*9 = 126 <= 128)
ROWS = GROUP * NB                 # 126 used partitions
QG = 8                            # groups per oct (two PSUM pages)
PAGES = QG // 4                   # PSUM pages per oct
QTOK = GROUP * QG                 # 112 tokens per oct
NQUAD = (TOK + QTOK - 1) // QTOK  # 10 (last oct ragged: 16 real tokens)

DVE_COPY_COLS = int(os.environ.get("BLOCKATTN_DVE_COPY", "560"))
ACT_SET = "natural_log_exp_and_others"

_CACHE = {}


def _groups(q):
    """[(g, t0, tg)] active groups of quad q (t0 = core-local token base)."""
    out = []
    for g in range(QG):
        t0 = q * QTOK + g * GROUP
        tg = min(GROUP, TOK - t0)
        if tg > 0:
            out.append((g, t0, tg))
    return out


def _patch_act_tables():
    """Make every activation func this kernel uses resolve to one table set
    (ACT_SET), so bacc emits a single ACT_TABLE_LOAD instead of thrashing
    between sets on every Ln/Exp/Square transition."""
    import concourse.bacc as bacc_mod
    import concourse.hw_specs as hw_specs
    from concourse import mybir

    if getattr(bacc_mod, "_blockattn_act_patch", False):
        return
    AF = mybir.ActivationFunctionType
    mine = {AF.Square, AF.Exp, AF.Ln, AF.Copy, AF.Identity}
    orig = hw_specs.get_activation_tables

    def patched(arch):
        t = dict(orig(arch))
        assert ACT_SET in t and mine <= t[ACT_SET], (ACT_SET, t.get(ACT_SET))
        return {
            name: (funcs if name == ACT_SET else funcs - mine)
            for name, funcs in t.items()
        }

    bacc_mod.get_activation_tables = patched
    bacc_mod._blockattn_act_patch = True


def build_nc():
    import concourse.bacc as bacc
    import concourse.tile as tile
    from concourse import mybir

    _patch_act_tables()

    f32 = mybir.dt.float32
    AF = mybir.ActivationFunctionType
    OP = mybir.AluOpType

    nc = bacc.Bacc("TRN2", target_bir_lowering=False, debug=False)

    vst_d = nc.dram_tensor("vstack", [NQUAD, ROWS, QG * D], f32,
                           kind="ExternalInput")
    w2b_d = nc.dram_tensor("w2b", [ROWS, D], f32, kind="ExternalInput")
    oh_d = nc.dram_tensor("onehot", [ROWS, GROUP], f32, kind="ExternalInput")
    ohT_d = nc.dram_tensor("onehotT", [GROUP, ROWS], f32, kind="ExternalInput")
    oh8_d = nc.dram_tensor("onehot8", [ROWS, QG * GROUP], f32, kind="ExternalInput")
    h_d = nc.dram_tensor("h", [TOK, D], f32, kind="ExternalOutput")

    vst = vst_d.ap()
    hout = h_d.ap()

    with tile.TileContext(nc) as tc:
        import contextlib
        ctx = contextlib.ExitStack()
        with ctx:
            consts = ctx.enter_context(tc.tile_pool(name="consts", bufs=1))
            vq_pool = ctx.enter_context(tc.tile_pool(name="vq", bufs=4))
            scr_pool = ctx.enter_context(tc.tile_pool(name="scr", bufs=2))
            stats_pool = ctx.enter_context(tc.tile_pool(name="stats", bufs=4))
            small_pool = ctx.enter_context(tc.tile_pool(name="small", bufs=3))
            hsb_pool = ctx.enter_context(tc.tile_pool(name="hsb", bufs=3))
            hpage_pool = ctx.enter_context(
                tc.tile_pool(name="hpage", bufs=3, space="PSUM"))
            zp_pool = ctx.enter_context(
                tc.tile_pool(name="zp", bufs=1, space="PSUM"))
            rzb_pool = ctx.enter_context(
                tc.tile_pool(name="rzb", bufs=1, space="PSUM"))

            w2b = consts.tile([ROWS, D], f32)
            nc.sync.dma_start(w2b[:], w2b_d.ap()[:])
            oh = consts.tile([ROWS, GROUP], f32)
            nc.sync.dma_start(oh[:], oh_d.ap()[:])
            ohT = consts.tile([GROUP, ROWS], f32)
            nc.sync.dma_start(ohT[:], ohT_d.ap()[:])
            oh8 = consts.tile([ROWS, QG * GROUP], f32)
            nc.sync.dma_start(oh8[:], oh8_d.ap()[:])
            zero_col = consts.tile([ROWS, 1], f32)
            nc.vector.memset(zero_col[:], 0.0)
            eps_col = consts.tile([ROWS, 1], f32)
            nc.vector.memset(eps_col[:], EPS)

            for q in range(NQUAD):
                groups = _groups(q)

                vq = vq_pool.tile([ROWS, QG * D], f32)
                stats = stats_pool.tile([ROWS, 2 * QG], f32)

                # ---- input DMA: one contiguous slab per oct ----
                # (tail oct: only transfer the columns of active groups)
                used = len(groups) * D
                nc.sync.dma_start(vq[:, 0:used], vst[q][:, 0:used])

                # ---- per-group stats (one full pass each on ACT and DVE) ----
                for g, t0, tg in groups:
                    gc = g * D
                    sq_scr = scr_pool.tile([ROWS, D], f32, tag="sq_scr")
                    nc.scalar.activation(
                        sq_scr[0:ROWS, :], vq[0:ROWS, gc:gc + D], AF.Square,
                        bias=zero_col[:], accum_out=stats[:, g:g + 1])
                    u_scr = scr_pool.tile([ROWS, D], f32, tag="u_scr")
                    nc.vector.scalar_tensor_tensor(
                        out=u_scr[0:ROWS, :], in0=vq[0:ROWS, gc:gc + D],
                        scalar=1.0, in1=w2b[0:ROWS, :],
                        op0=OP.mult, op1=OP.mult,
                        accum_out=stats[:, QG + g:QG + g + 1])

                # ---- softmax small ops on [126, 4] stats ----
                lnq = small_pool.tile([ROWS, QG], f32, tag="lnq")
                nc.scalar.activation(lnq[:], stats[:, 0:QG], AF.Ln,
                                     bias=eps_col[:], scale=1.0 / D)
                rq = small_pool.tile([ROWS, QG], f32, tag="rq")
                nc.scalar.activation(rq[:], lnq[:], AF.Exp,
                                     bias=zero_col[:], scale=-0.5)
                lg = small_pool.tile([ROWS, QG], f32, tag="lg")
                nc.vector.tensor_mul(lg[:], stats[:, QG:2 * QG], rq[:])
                e_sb = small_pool.tile([ROWS, QG], f32, tag="e_sb")
                nc.scalar.activation(e_sb[:], lg[:], AF.Exp, bias=zero_col[:])

                zp = zp_pool.tile([GROUP, QG], f32)
                nc.tensor.matmul(zp[:], lhsT=oh[:], rhs=e_sb[:],
                                 start=True, stop=True)
                rz = small_pool.tile([GROUP, QG], f32, tag="rz")
                nc.vector.reciprocal(rz[:], zp[:])
                rzb = rzb_pool.tile([ROWS, QG], f32)
                nc.tensor.matmul(rzb[:], lhsT=ohT[:], rhs=rz[:],
                                 start=True, stop=True)
                wcol = small_pool.tile([ROWS, QG], f32, tag="wcol")
                nc.vector.tensor_mul(wcol[:], e_sb[:], rzb[:])

                # ---- weighted sum via PE, 4 groups per PSUM page ----
                lhsTs = small_pool.tile([ROWS, QG * GROUP], f32, tag="lhsTs")
                active_pages = sorted({g // 4 for g, _, _ in groups})
                hpages = {pg: hpage_pool.tile([128, D], f32, tag="hpage",
                                              name="hpage")
                          for pg in active_pages}
                nc.vector.tensor_tensor(
                    out=lhsTs[:, :].rearrange("p (g j) -> p g j", g=QG),
                    in0=oh8[:, :].rearrange("p (g j) -> p g j", g=QG),
                    in1=wcol[:, :].unsqueeze(2).to_broadcast(
                        [ROWS, QG, GROUP]),
                    op=OP.mult)
                for g, t0, tg in groups:
                    gc = g * D
                    lw = lhsTs[:, g * GROUP:(g + 1) * GROUP]
                    pg = g // 4
                    col = 32 * (g % 4)
                    for hh in range(2):
                        nc.tensor.matmul(
                            hpages[pg][col:col + GROUP,
                                       512 * hh:512 * hh + 512],
                            lhsT=lw,
                            rhs=vq[0:ROWS, gc + 512 * hh:gc + 512 * hh + 512],
                            start=True, stop=True,
                            tile_position=(0, col))

                # ---- PSUM -> SBUF (split across DVE and ACT) -> HBM ----
                for pg in active_pages:
                    h_sb = hsb_pool.tile([128, D], f32, tag="h_sb")
                    nc.vector.tensor_copy(h_sb[:, 0:DVE_COPY_COLS],
                                          hpages[pg][:, 0:DVE_COPY_COLS])
                    nc.scalar.copy(h_sb[:, DVE_COPY_COLS:D],
                                   hpages[pg][:, DVE_COPY_COLS:D])
                    for g, t0, tg in groups:
                        if g // 4 != pg:
                            continue
                        nc.gpsimd.dma_start(hout[t0:t0 + tg, :],
                                            h_sb[32 * (g % 4):32 * (g % 4) + tg, :])

    nc.compile()
    return nc


def _host_inputs(blocks, partial_block, proj_w, norm_w):
    """Slice + interleave per-core inputs (host-side, numpy only)."""
    blocks = np.ascontiguousarray(blocks, dtype=np.float32).reshape(N, B * T, D)
    partial = np.ascontiguousarray(partial_block, dtype=np.float32).reshape(B * T, D)
    w2 = (np.asarray(proj_w, np.float32) * np.asarray(norm_w, np.float32))
    w2b = np.ascontiguousarray(np.broadcast_to(w2, (ROWS, D)), np.float32)
    oh = np.zeros((ROWS, GROUP), np.float32)
    for p in range(ROWS):
        oh[p, p % GROUP] = 1.0
    ohT = np.ascontiguousarray(oh.T)
    oh8 = np.ascontiguousarray(np.tile(oh, (1, QG)))

    pad_tok = NQUAD * QTOK  # 1064
    in_maps = []
    for c in range(N_CORES):
        s = slice(c * TOK, (c + 1) * TOK)
        av = np.zeros((NB, pad_tok, D), np.float32)
        av[:N, :TOK] = blocks[:, s, :]
        av[N, :TOK] = partial[s, :]
        # vstack[q, 14n+t', g*D+d] = av[n, q*56 + g*14 + t', d]
        vst = av.reshape(NB, NQUAD, QG, GROUP, D)
        vst = np.ascontiguousarray(vst.transpose(1, 0, 3, 2, 4))
        vst = vst.reshape(NQUAD, ROWS, QG * D)
        in_maps.append({
            "vstack": vst,
            "w2b": w2b,
            "onehot": oh,
            "onehotT": ohT,
            "onehot8": oh8,
        })
    return in_maps


def kernel(blocks, partial_block, proj_w, norm_w):
    from concourse.bass_utils import run_bass_kernel_spmd

    if "nc" not in _CACHE:
        _CACHE["nc"] = build_nc()
    nc = _CACHE["nc"]
    in_maps = _host_inputs(blocks, partial_block, proj_w, norm_w)
    res = run_bass_kernel_spmd(nc, in_maps, core_ids=list(range(N_CORES)))
    h = np.concatenate([res.results[c]["h"] for c in range(N_CORES)], axis=0)
    return h.reshape(B, T, D)
